# revision 9
# baseline (speedup 1.0000x reference)
"""Stress-majorization loss kernel for Trainium2 (8 NeuronCores).

Problem: pos [8192,2] f32, dist [8192,8192] f32 ->
    scalar sum of ((|p_i - p_j| - d_ij)/d_ij)^2 over entries with d_ij != 0.

Strategy (per-core row sharding, 1024 rows each):
 - Host: ship r = 1/d in bf16 (r = 0 for d==0 entries -> each contributes
   exactly (0-1)^2 = 1, removed via the host-side zero count).  Halves HBM
   traffic vs shipping d (16MB/core) and removes the device reciprocal
   pass.  Squared pairwise distances factor as a K=24 bf16 matmul.
 - Device, per [128,8192] row-tile, chunked [2048,1536x4] to fit PSUM:
     PE:  sq -> PSUM f32 (matmuls of 512 cols, K=24 bf16)
     ACT: pred = sqrt(psum) -> bf16 SBUF     (the only ACT pass)
     DVE: w = pred * r  via tensor_tensor_reduce (2x bf16) which also
          emits accum_out = sum(w) per partition -> Sigma_w for free
     square-reduce, split for engine balance using
          sum((w-1)^2) = sum(w^2) - 2*sum(w) + M:
       chunks 0,2,4: PE gram accumulate w_blk^T @ w_blk (128-col blocks)
          into one PSUM bank; diag of the final [128,128] = sum(w^2)
       chunks 1,3:   DVE in-place tensor_tensor_reduce (w*w, accum add)
 - PE warm-up: ~40 back-to-back tiny matmuls at kernel start keep the HAM
   clock gate at 8/8 (2.4 GHz); pipeline gaps stay < 3.4us so it never
   re-throttles.  Gram matmuls are emitted 3 chunks behind the sq matmuls
   so the in-order PE never waits on the DVE chain.
 - Host: total = sum_cores [trace(gram) + Sigma_w2_dve - 2*Sigma_w]
                 + N^2 - (#zeros in dist).
"""
import sys
sys.path.insert(0, "/opt/trn_rl_repo")

import numpy as np
import ml_dtypes

N = 8192
NCORES = 8
ROWS_PER_CORE = N // NCORES          # 1024
RTILES = ROWS_PER_CORE // 128        # 8 row tiles of 128
# chunk layout per row-tile: 4+3 PSUM banks double-buffered + 1 gram bank
CHUNKS = [(0, 2048), (2048, 1536), (3584, 1536), (5120, 1536), (6656, 1536)]
DVE_SQ_CHUNKS = {1, 3}               # square-reduce on DVE for these chunks
GRAM_LAG = 3                         # emit gram MMs this many chunks behind
WARMUP_MMS = 100                     # tiny MMs to trip the HAM clock gate
                                     # (~53ns issue each; need >3.4us busy)
KB = 4                               # base contraction dim
NPAIR = 6                            # bf16 split term-pairs kept
K = KB * NPAIR                       # 24
EPS = np.float32(4e-6)               # keeps PSUM sq > 0 despite cancellation

NW_CHUNKS = RTILES * len(CHUNKS)                       # 40 w-accum cols
NSQ_CHUNKS = RTILES * len(DVE_SQ_CHUNKS)               # 16 w2-accum cols
ACC_COLS = NW_CHUNKS + NSQ_CHUNKS                      # 56

# "ttv": TT mult + TS sub + gram on v for all chunks (host: trace - nzeros)
# "ttr": TTR (w + sum_w) + gram on w / TTR^2 split (host: moment formula)
VARIANT = "ttv"
WARMUP = True

_cache = {}


def _build_nc():
    import concourse.bacc as bacc
    import concourse.mybir as mybir
    import concourse.tile as tile

    f32 = mybir.dt.float32
    bf16 = mybir.dt.bfloat16
    A = mybir.ActivationFunctionType
    OP = mybir.AluOpType

    nc = bacc.Bacc("TRN2", target_bir_lowering=False, debug=False)
    rdist = nc.dram_tensor("rdist", [ROWS_PER_CORE, N], bf16, kind="ExternalInput")
    acore = nc.dram_tensor("acore", [K, ROWS_PER_CORE], bf16, kind="ExternalInput")
    bfull = nc.dram_tensor("bfull", [K, N], bf16, kind="ExternalInput")
    gout = nc.dram_tensor("gram", [128, 128], f32, kind="ExternalOutput")
    aout = nc.dram_tensor("acc", [128, ACC_COLS], f32, kind="ExternalOutput")

    if VARIANT == "ttv":
        gram_chunks = list(range(len(CHUNKS)))
    else:
        gram_chunks = [ci for ci in range(len(CHUNKS)) if ci not in DVE_SQ_CHUNKS]
    nblocks_total = RTILES * sum(CHUNKS[ci][1] for ci in gram_chunks) // 128

    with tile.TileContext(nc) as tc:
        with tc.tile_pool(name="small", bufs=1) as small, \
             tc.tile_pool(name="rpool", bufs=6) as rpool, \
             tc.tile_pool(name="prpool", bufs=2) as prpool, \
             tc.tile_pool(name="psA", bufs=1, space="PSUM") as psA, \
             tc.tile_pool(name="psB", bufs=1, space="PSUM") as psB, \
             tc.tile_pool(name="psG", bufs=1, space="PSUM") as psG:

            t_a = small.tile([K, ROWS_PER_CORE], bf16)
            t_b = small.tile([K, N], bf16)
            t_g = small.tile([128, 128], f32)
            t_acc = small.tile([128, ACC_COLS], f32)
            nc.sync.dma_start(t_a[:], acore[:])
            nc.sync.dma_start(t_b[:], bfull[:])
            nc.vector.memset(t_acc[:], 0.0)

            gps = psG.tile([128, 128], f32, tag="g")

            if WARMUP:
                # HAM warm-up: ~40 back-to-back tiny matmuls (~4us cold) flip
                # the PE clock gate to 8/8 before the real pipeline starts.
                for _ in range(WARMUP_MMS):
                    nc.tensor.matmul(gps[:64, :64], t_a[:, :64], t_b[:, :64],
                                     start=True, stop=True)

            pending = []                 # (pred_tile, c0, wc) gram work queue
            blk = [0]

            def emit_gram(pred_t, c0, wc):
                for b in range(wc // 128):
                    s = c0 + b * 128
                    nc.tensor.matmul(
                        gps[:], pred_t[:, s:s + 128], pred_t[:, s:s + 128],
                        start=(blk[0] == 0), stop=(blk[0] == nblocks_total - 1))
                    blk[0] += 1

            iw = [0]
            isq = [0]
            for rt in range(RTILES):
                lhsT = t_a[:, rt * 128:(rt + 1) * 128]
                pred = prpool.tile([128, N], bf16, tag="pred")
                for ci, (c0, wc) in enumerate(CHUNKS):
                    t_r = rpool.tile([128, wc], bf16, tag="r")
                    nc.sync.dma_start(
                        t_r[:], rdist[rt * 128:(rt + 1) * 128, c0:c0 + wc])
                    pool = psA if ci % 2 == 0 else psB
                    ps = pool.tile([128, wc], f32, tag="psq")
                    for j in range(wc // 512):
                        nc.tensor.matmul(
                            ps[:, j * 512:(j + 1) * 512], lhsT,
                            t_b[:, c0 + j * 512:c0 + (j + 1) * 512],
                            start=True, stop=True)
                    nc.scalar.activation(pred[:, c0:c0 + wc], ps[:], A.Sqrt)
                    if VARIANT == "ttv":
                        nc.vector.tensor_tensor(
                            pred[:, c0:c0 + wc], pred[:, c0:c0 + wc], t_r[:],
                            OP.mult)
                        nc.vector.tensor_scalar(
                            out=pred[:, c0:c0 + wc], in0=pred[:, c0:c0 + wc],
                            scalar1=1.0, scalar2=None, op0=OP.subtract)
                        pending.append((pred, c0, wc))
                        if len(pending) > GRAM_LAG:
                            emit_gram(*pending.pop(0))
                        continue
                    # w = pred * r, and accum_out = sum(w) per partition
                    nc.vector.tensor_tensor_reduce(
                        out=pred[:, c0:c0 + wc],
                        in0=pred[:, c0:c0 + wc], in1=t_r[:],
                        scale=1.0, scalar=0.0,
                        op0=OP.mult, op1=OP.add,
                        accum_out=t_acc[:, iw[0]:iw[0] + 1])
                    iw[0] += 1
                    if ci in DVE_SQ_CHUNKS:
                        # in-place w*w with accum -> sum(w^2) for this chunk
                        nc.vector.tensor_tensor_reduce(
                            out=pred[:, c0:c0 + wc],
                            in0=pred[:, c0:c0 + wc], in1=pred[:, c0:c0 + wc],
                            scale=1.0, scalar=0.0,
                            op0=OP.mult, op1=OP.add,
                            accum_out=t_acc[:, NW_CHUNKS + isq[0]:
                                            NW_CHUNKS + isq[0] + 1])
                        isq[0] += 1
                    else:
                        pending.append((pred, c0, wc))
                        if len(pending) > GRAM_LAG:
                            emit_gram(*pending.pop(0))
            while pending:
                emit_gram(*pending.pop(0))

            nc.scalar.copy(t_g[:], gps[:])
            nc.sync.dma_start(gout[:], t_g[:])
            nc.sync.dma_start(aout[:], t_acc[:])

    nc.compile()
    return nc


def _split3(v: np.ndarray):
    """Split fp32 vector into 3 bf16 terms summing to v (error ~2^-27 |v|)."""
    v = v.astype(np.float32)
    v0 = v.astype(ml_dtypes.bfloat16)
    r1 = v - v0.astype(np.float32)
    v1 = r1.astype(ml_dtypes.bfloat16)
    r2 = r1 - v1.astype(np.float32)
    v2 = r2.astype(ml_dtypes.bfloat16)
    return v0, v1, v2


def _to_np_f32(x):
    try:
        return np.ascontiguousarray(x, dtype=np.float32)
    except Exception:
        import jax
        return np.ascontiguousarray(jax.device_get(x), dtype=np.float32)


def _prep_inputs(pos: np.ndarray, dist: np.ndarray):
    pos = _to_np_f32(pos)
    dist = _to_np_f32(dist)
    assert pos.shape == (N, 2) and dist.shape == (N, N)

    # host-side prep: r = 1/d in bf16; r=0 for masked (d==0) entries so the
    # device yields exactly (0-1)^2 = 1 there (removed via nzeros below)
    zmask = dist == 0.0
    nzeros = int(np.count_nonzero(zmask))
    with np.errstate(divide="ignore"):
        r = np.where(zmask, np.float32(0.0), np.float32(1.0) / dist)
    r16 = r.astype(ml_dtypes.bfloat16)

    x = pos[:, 0].astype(np.float64)
    y = pos[:, 1].astype(np.float64)
    n = x * x + y * y
    a_full32 = np.stack([np.ones(N), n + np.float64(EPS), -2.0 * x, -2.0 * y]
                        ).astype(np.float32)          # [4, N]
    b_full32 = np.stack([n, np.ones(N), x, y]).astype(np.float32)  # [4, N]

    a0, a1, a2 = _split3(a_full32)
    b0, b1, b2 = _split3(b_full32)
    # term pairs kept: (a0,b0) (a0,b1) (a1,b0) (a0,b2) (a2,b0) (a1,b1)
    a_parts = [a0, a0, a1, a0, a2, a1]
    b_parts = [b0, b1, b0, b2, b0, b1]
    a_full = np.concatenate(a_parts, axis=0)   # [24, N] bf16
    b_full = np.concatenate(b_parts, axis=0)   # [24, N] bf16

    in_maps = []
    for c in range(NCORES):
        r0 = c * ROWS_PER_CORE
        in_maps.append({
            "rdist": np.ascontiguousarray(r16[r0:r0 + ROWS_PER_CORE, :]),
            "acore": np.ascontiguousarray(a_full[:, r0:r0 + ROWS_PER_CORE]),
            "bfull": b_full,
        })
    return in_maps, nzeros


def kernel(pos: np.ndarray, dist: np.ndarray) -> np.ndarray:
    from concourse.bass_utils import run_bass_kernel_spmd

    in_maps, nzeros = _prep_inputs(pos, dist)
    if "nc" not in _cache:
        _cache["nc"] = _build_nc()
    nc = _cache["nc"]

    res = run_bass_kernel_spmd(nc, in_maps, list(range(NCORES)))
    if VARIANT == "ttv":
        total = -float(nzeros)
        for c in range(NCORES):
            total += np.trace(res.results[c]["gram"].astype(np.float64))
        return np.array(total, dtype=np.float32)
    total = float(N) * float(N) - float(nzeros)
    for c in range(NCORES):
        g = res.results[c]["gram"].astype(np.float64)
        acc = res.results[c]["acc"].astype(np.float64)
        s_w = acc[:, :NW_CHUNKS].sum()
        s_w2_dve = acc[:, NW_CHUNKS:].sum()
        total += np.trace(g) + s_w2_dve - 2.0 * s_w
    return np.array(total, dtype=np.float32)
